# revision 1
# baseline (speedup 1.0000x reference)
"""Trainium2 Bass kernel for nn_Disentangler (ragged_sequence).

Math (per component-MLP g of 32; 16 node + 16 edge):
    rows  = x[mask]                      # [32768, 2048], row-major
    sel   = rows[idx_g]                  # [1000, 2048]
    h     = gelu(sel @ W1_g + b1_g)      # [1000, 4096]
    y     = h @ W2_g + b2_g              # [1000, 1024]
    pooled= segsum_dedup(y) / TOK        # [16, 1024]
Key folding: the scatter+segment_sum+mean is a linear map P_g ([16,1000],
entries 1/TOK at first-occurrence sample positions, bucketed by timestamp),
so  pooled = (P_g @ h) @ W2_g + (P_g 1) b2_g.  This replaces the big second
matmul [1000x4096x1024] with a tiny pooling matmul [16x1000x4096] plus
[16x4096x1024], cutting FLOPs/component from 25.2G to ~17.1G.

Distribution: expert parallelism — 4 components per NeuronCore (8 cores),
no collectives. The host gathers sel^T per component (the only part of x a
core needs), builds P_g (dedup + /TOK baked in), and feeds per-core weights.

On-chip layout (per component):
    MM1:  h^T-free orientation: h[125m, 256nb] = selT[:,m]^T @ W1[:, nb]
          fp32r matmuls, K=2048 as 16 chunks on partitions, PSUM-accumulated.
    gelu: ScalarE exact Gelu, PSUM -> SBUF.
    pool: poolPh[16, 256nb] = sum_m PT[:,m]^T @ h_m  (PE, PSUM-accumulated)
    transpose: PE-transpose poolPh 128-col slices -> poolPhT [128, 16] chunks
    miniW2: pooled[16,1024] += poolPhT_chunk^T @ W2_chunk, accumulated across
          n-blocks via DVE adds into an SBUF accumulator.
b1/b2 are structurally zero for this problem; b2 != 0 would be corrected on
the host (linear), b1 != 0 is rejected.
"""

import numpy as np

import concourse.bacc as bacc
import concourse.mybir as mybir
from concourse.tile import TileContext
from concourse.bass_utils import run_bass_kernel_spmd

F32R = mybir.dt.float32r
F32 = mybir.dt.float32
GELU = mybir.ActivationFunctionType.Gelu

T, TOK, D = 16, 4096, 2048
C = D // 2            # 1024
H = 2 * C             # 4096
K = 16                # components per branch
L = 1000              # samples per component
NN = T * (TOK // 2)   # 32768 rows per branch
NCORES = 8
CPC = (2 * K) // NCORES   # components per core = 4

P = 128
KD = D // P           # 16 contraction chunks for MM1
KH = H // P           # 32 chunks over H
NBW = 256             # H-columns per n-block
NB = H // NBW         # 16 n-blocks
MT = 8                # row chunks
MW = L // MT          # 125 rows per chunk

SELQ = 4              # selT split into 4 quarters of 4 k-chunks each
SELQ_BUFS = 7


def build_nc():
    nc = bacc.Bacc(None)
    selT = nc.dram_tensor("selT", [CPC, KD, P, L], F32R, kind="ExternalInput")
    w1 = nc.dram_tensor("w1", [CPC, KD, P, H], F32R, kind="ExternalInput")
    w2 = nc.dram_tensor("w2", [CPC, KH, P, C], F32R, kind="ExternalInput")
    pt = nc.dram_tensor("pt", [CPC, MW, MT, 16], F32R, kind="ExternalInput")
    out = nc.dram_tensor("out", [CPC, T, C], F32, kind="ExternalOutput")

    with TileContext(nc) as tc:
        with (
            tc.tile_pool(name="selp", bufs=SELQ_BUFS) as selp,
            tc.tile_pool(name="w1p", bufs=2) as w1p,
            tc.tile_pool(name="w2p", bufs=2) as w2p,
            tc.tile_pool(name="hp", bufs=MT + 2) as hp,
            tc.tile_pool(name="smallp", bufs=2) as smallp,
            tc.tile_pool(name="outp", bufs=2) as outp,
            tc.tile_pool(name="cstp", bufs=1) as cstp,
            tc.tile_pool(name="ps_h", bufs=2, space="PSUM") as ps_h,
            tc.tile_pool(name="ps_pool", bufs=2, space="PSUM") as ps_pool,
            tc.tile_pool(name="ps_tr", bufs=1, space="PSUM") as ps_tr,
            tc.tile_pool(name="ps_out", bufs=1, space="PSUM") as ps_out,
        ):
            ident = cstp.tile([16, 16], F32R, tag="ident")
            ident_dram = nc.inline_tensor(np.eye(16, dtype=np.float32), "ident16")
            nc.sync.dma_start(ident[:], ident_dram[:].bitcast(F32R))

            for c in range(CPC):
                sel_sb = []
                for q in range(SELQ):
                    tq = selp.tile([P, KD // SELQ, L], F32R, tag="selq")
                    nc.sync.dma_start(
                        tq[:],
                        selT[c, 4 * q : 4 * q + 4].rearrange("ko ki l -> ki ko l"),
                    )
                    sel_sb.append(tq)
                pt_sb = smallp.tile([MW, MT, 16], F32R, tag="pt")
                nc.sync.dma_start(pt_sb[:], pt[c])
                out_sb = outp.tile([T, C], F32, tag="out")

                for nb in range(NB):
                    w1_sb = w1p.tile([P, KD, NBW], F32R, tag="w1")
                    nc.sync.dma_start(
                        w1_sb[:],
                        w1[c, :, :, nb * NBW : (nb + 1) * NBW].rearrange(
                            "ko ki n -> ki ko n"
                        ),
                    )
                    w2_sb = w2p.tile([P, 2, C], F32R, tag="w2")
                    nc.sync.dma_start(
                        w2_sb[:],
                        w2[c, 2 * nb : 2 * nb + 2].rearrange("ko ki n -> ki ko n"),
                    )

                    # --- MM1 + gelu for all row chunks of this n-block ---
                    h_tiles = []
                    for m in range(MT):
                        h_ps = ps_h.tile([MW, NBW], F32, tag="hps")
                        for k in range(KD):
                            nc.tensor.matmul(
                                h_ps[:],
                                sel_sb[k // 4][:, k % 4, m * MW : (m + 1) * MW],
                                w1_sb[:, k],
                                start=(k == 0),
                                stop=(k == KD - 1),
                            )
                        h_sb = hp.tile([MW, NBW], F32R, tag="h")
                        nc.scalar.activation(h_sb[:], h_ps[:], GELU)
                        h_tiles.append(h_sb)

                    # --- pooling matmul, accumulated over row chunks ---
                    pool_ps = ps_pool.tile([16, NBW], F32, tag="poolps")
                    for m in range(MT):
                        nc.tensor.matmul(
                            pool_ps[:],
                            pt_sb[:, m],
                            h_tiles[m][:],
                            start=(m == 0),
                            stop=(m == MT - 1),
                        )
                    pp_sb = smallp.tile([16, NBW], F32R, tag="pp")
                    nc.vector.tensor_copy(pp_sb[:], pool_ps[:].bitcast(F32R))

                    # --- PE-transpose poolPh 128-col slices -> [128,16] ---
                    tr_ps = ps_tr.tile([P, 32], F32, tag="trps")
                    for i in range(2):
                        nc.tensor.transpose(
                            tr_ps[:, 16 * i : 16 * (i + 1)].bitcast(F32R),
                            pp_sb[:, P * i : P * (i + 1)],
                            ident[:],
                        )
                    pht_sb = smallp.tile([P, 32], F32R, tag="pht")
                    nc.vector.tensor_copy(pht_sb[:], tr_ps[:].bitcast(F32R))

                    # --- miniW2: pooled += poolPhT_chunk^T @ W2_chunk ---
                    po_ps = ps_out.tile([16, C], F32, tag="pops")
                    for hh in range(2):
                        for i in range(2):
                            nc.tensor.matmul(
                                po_ps[:, 512 * hh : 512 * (hh + 1)],
                                pht_sb[:, 16 * i : 16 * (i + 1)],
                                w2_sb[:, i, 512 * hh : 512 * (hh + 1)],
                                start=(i == 0),
                                stop=(i == 1),
                            )
                    if nb == 0:
                        nc.vector.tensor_copy(out_sb[:], po_ps[:])
                    else:
                        nc.vector.tensor_add(out_sb[:], out_sb[:], po_ps[:])

                nc.sync.dma_start(out[c], out_sb[:])

    nc.finalize()
    return nc


_CACHED_NC = None


def _get_nc():
    global _CACHED_NC
    if _CACHED_NC is None:
        _CACHED_NC = build_nc()
    return _CACHED_NC


def prepare_inputs(inputs):
    """Host-side sharding: gather selT, build pooling matrices, view weights.

    Returns (in_maps, b2_corrections) where b2_corrections[g] is the host-side
    rank-1 term cnt_t (x) b2_g / TOK to add for nonzero b2 (zero here).
    """
    x = np.ascontiguousarray(np.asarray(inputs["x"], dtype=np.float32))
    nm = np.asarray(inputs["padded_node_mask"])
    em = np.asarray(inputs["padded_edge_mask"])
    ridx = np.asarray(inputs["rand_indices"])

    node_W1 = np.asarray(inputs["node_W1"], dtype=np.float32)
    node_W2 = np.asarray(inputs["node_W2"], dtype=np.float32)
    edge_W1 = np.asarray(inputs["edge_W1"], dtype=np.float32)
    edge_W2 = np.asarray(inputs["edge_W2"], dtype=np.float32)
    for bname in ("node_b1", "node_b2", "edge_b1", "edge_b2"):
        b = np.asarray(inputs[bname])
        if bname.endswith("b1") and np.any(b):
            raise NotImplementedError("nonzero b1 not supported by this kernel")

    xf = x.reshape(T * TOK, D)
    nt, ntok = np.nonzero(nm)
    et, etok = np.nonzero(em)
    assert nt.size == NN and et.size == NN, "unexpected mask population"
    flat_n = nt * TOK + ntok
    flat_e = et * TOK + etok

    in_maps = []
    b2_corr = np.zeros((2 * K, T, C), np.float32)
    any_b2 = np.any(inputs["node_b2"]) or np.any(inputs["edge_b2"])
    for core in range(NCORES):
        sel_list, pt_list = [], []
        for j in range(CPC):
            g = core * CPC + j
            if g < K:
                flat, seg, b2 = flat_n, nt, np.asarray(inputs["node_b2"])[g]
            else:
                flat, seg, b2 = flat_e, et, np.asarray(inputs["edge_b2"])[g - K]
            idx = ridx[g]
            selT_g = np.ascontiguousarray(xf[flat[idx]].T).reshape(KD, P, L)
            sel_list.append(selT_g)
            pt_mat = np.zeros((L, 16), np.float32)
            _, first = np.unique(idx, return_index=True)
            tvals = seg[idx[first]]
            pt_mat[first, tvals] = 1.0 / TOK
            pt_list.append(pt_mat.reshape(MT, MW, 16).transpose(1, 0, 2))
            if any_b2:
                cnt = np.bincount(tvals, minlength=T).astype(np.float32)
                b2_corr[g] = np.outer(cnt / TOK, b2.astype(np.float32))
        if core * CPC < K:
            w1v = node_W1[core * CPC : core * CPC + CPC].reshape(CPC, KD, P, H)
            w2v = node_W2[core * CPC : core * CPC + CPC].reshape(CPC, KH, P, C)
        else:
            o = core * CPC - K
            w1v = edge_W1[o : o + CPC].reshape(CPC, KD, P, H)
            w2v = edge_W2[o : o + CPC].reshape(CPC, KH, P, C)
        in_maps.append(
            {
                "selT": np.ascontiguousarray(np.stack(sel_list)),
                "w1": w1v,
                "w2": w2v,
                "pt": np.ascontiguousarray(np.stack(pt_list)),
            }
        )
    return in_maps, b2_corr


def assemble_output(results, b2_corr):
    comp_all = np.empty((2 * K, T, C), np.float32)
    for core in range(NCORES):
        comp_all[core * CPC : (core + 1) * CPC] = results[core]["out"]
    comp_all += b2_corr
    return np.ascontiguousarray(comp_all.transpose(1, 0, 2).reshape(T, 1, 2 * K * C))


def kernel(**inputs) -> np.ndarray:
    in_maps, b2_corr = prepare_inputs(inputs)
    nc = _get_nc()
    res = run_bass_kernel_spmd(nc, in_maps, list(range(NCORES)))
    return assemble_output(res.results, b2_corr)


# revision 3
# speedup vs baseline: 48.0967x; 48.0967x over previous
"""Trainium2 Bass kernel for nn_Disentangler (ragged_sequence).

Math (per component-MLP g of 32; 16 node + 16 edge):
    rows  = x[mask]                      # [32768, 2048], row-major
    sel   = rows[idx_g]                  # [1000, 2048]
    h     = gelu(sel @ W1_g + b1_g)      # [1000, 4096]
    y     = h @ W2_g + b2_g              # [1000, 1024]
    pooled= segsum_dedup(y) / TOK        # [16, 1024]
Key folding: the scatter+segment_sum+mean is a linear map P_g ([16,1000],
entries 1/TOK at first-occurrence sample positions, bucketed by timestamp),
so  pooled = (P_g @ h) @ W2_g + (P_g 1) b2_g.  This replaces the big second
matmul [1000x4096x1024] with a tiny pooling matmul [16x1000x4096] plus
[16x4096x1024], cutting FLOPs/component from 25.2G to ~17.1G.

Distribution: expert parallelism — 4 components per NeuronCore (8 cores),
no collectives. The host gathers sel^T per component (the only part of x a
core needs), builds P_g (dedup + /TOK baked in), and feeds per-core weights.

On-chip layout (per component):
    MM1:  h^T-free orientation: h[125m, 256nb] = selT[:,m]^T @ W1[:, nb]
          fp32r matmuls, K=2048 as 16 chunks on partitions, PSUM-accumulated.
    gelu: ScalarE exact Gelu, PSUM -> SBUF.
    pool: poolPh[16, 256nb] = sum_m PT[:,m]^T @ h_m  (PE, PSUM-accumulated)
    transpose: PE-transpose poolPh 128-col slices -> poolPhT [128, 16] chunks
    miniW2: pooled[16,1024] += poolPhT_chunk^T @ W2_chunk, accumulated across
          n-blocks via DVE adds into an SBUF accumulator.
b1/b2 are structurally zero for this problem; b2 != 0 would be corrected on
the host (linear), b1 != 0 is rejected.
"""

import numpy as np

import concourse.bacc as bacc
import concourse.mybir as mybir
from concourse.tile import TileContext
from concourse.bass_utils import run_bass_kernel_spmd

F32R = mybir.dt.float32r
F32 = mybir.dt.float32
GELU = mybir.ActivationFunctionType.Gelu

T, TOK, D = 16, 4096, 2048
C = D // 2            # 1024
H = 2 * C             # 4096
K = 16                # components per branch
L = 1000              # samples per component
NN = T * (TOK // 2)   # 32768 rows per branch
NCORES = 8
CPC = (2 * K) // NCORES   # components per core = 4

P = 128
KD = D // P           # 16 contraction chunks for MM1
KH = H // P           # 32 chunks over H
NBW = 256             # H-columns per n-block
NB = H // NBW         # 16 n-blocks
MT = 8                # row chunks
MW = L // MT          # 125 rows per chunk

SELQ = 4              # selT split into 4 quarters of 4 k-chunks each
SELQ_BUFS = 7


def build_nc(repeat=1):
    """repeat>1 re-emits the whole compute body; used only for timing
    (slope of wall-clock vs repeat cancels fixed dispatch overheads)."""
    nc = bacc.Bacc(None)
    selT = nc.dram_tensor("selT", [CPC, KD, P, L], F32R, kind="ExternalInput")
    w1 = nc.dram_tensor("w1", [CPC, KD, P, H], F32R, kind="ExternalInput")
    w2 = nc.dram_tensor("w2", [CPC, KH, P, C], F32R, kind="ExternalInput")
    pt = nc.dram_tensor("pt", [CPC, MW, MT, 16], F32R, kind="ExternalInput")
    out = nc.dram_tensor("out", [CPC, T, C], F32, kind="ExternalOutput")

    with TileContext(nc) as tc:
        with (
            tc.tile_pool(name="selp", bufs=SELQ_BUFS) as selp,
            tc.tile_pool(name="w1p", bufs=2) as w1p,
            tc.tile_pool(name="w2p", bufs=2) as w2p,
            tc.tile_pool(name="hp", bufs=MT + 2) as hp,
            tc.tile_pool(name="smallp", bufs=2) as smallp,
            tc.tile_pool(name="outp", bufs=2) as outp,
            tc.tile_pool(name="cstp", bufs=1) as cstp,
            tc.tile_pool(name="ps_h", bufs=2, space="PSUM") as ps_h,
            tc.tile_pool(name="ps_pool", bufs=2, space="PSUM") as ps_pool,
            tc.tile_pool(name="ps_tr", bufs=1, space="PSUM") as ps_tr,
            tc.tile_pool(name="ps_out", bufs=1, space="PSUM") as ps_out,
        ):
            ident = cstp.tile([16, 16], F32R, tag="ident")
            ident_dram = nc.inline_tensor(np.eye(16, dtype=np.float32), "ident16")
            nc.sync.dma_start(ident[:], ident_dram[:].bitcast(F32R))

            for c_rep in range(repeat * CPC):
                c = c_rep % CPC
                sel_sb = []
                for q in range(SELQ):
                    tq = selp.tile([P, KD // SELQ, L], F32R, tag="selq")
                    nc.sync.dma_start(
                        tq[:],
                        selT[c, 4 * q : 4 * q + 4].rearrange("ko ki l -> ki ko l"),
                    )
                    sel_sb.append(tq)
                pt_sb = smallp.tile([MW, MT, 16], F32R, tag="pt")
                nc.sync.dma_start(pt_sb[:], pt[c])
                out_sb = outp.tile([T, C], F32, tag="out")

                for nb in range(NB):
                    w1_sb = w1p.tile([P, KD, NBW], F32R, tag="w1")
                    nc.sync.dma_start(
                        w1_sb[:],
                        w1[c, :, :, nb * NBW : (nb + 1) * NBW].rearrange(
                            "ko ki n -> ki ko n"
                        ),
                    )
                    w2_sb = w2p.tile([P, 2, C], F32R, tag="w2")
                    nc.sync.dma_start(
                        w2_sb[:],
                        w2[c, 2 * nb : 2 * nb + 2].rearrange("ko ki n -> ki ko n"),
                    )

                    # --- MM1 + gelu for all row chunks of this n-block ---
                    h_tiles = []
                    for m in range(MT):
                        h_ps = ps_h.tile([MW, NBW], F32, tag="hps")
                        for k in range(KD):
                            nc.tensor.matmul(
                                h_ps[:],
                                sel_sb[k // 4][:, k % 4, m * MW : (m + 1) * MW],
                                w1_sb[:, k],
                                start=(k == 0),
                                stop=(k == KD - 1),
                            )
                        h_sb = hp.tile([MW, NBW], F32R, tag="h")
                        nc.scalar.activation(h_sb[:], h_ps[:], GELU)
                        h_tiles.append(h_sb)

                    # --- pooling matmul, accumulated over row chunks ---
                    pool_ps = ps_pool.tile([16, NBW], F32, tag="poolps")
                    for m in range(MT):
                        nc.tensor.matmul(
                            pool_ps[:],
                            pt_sb[:, m],
                            h_tiles[m][:],
                            start=(m == 0),
                            stop=(m == MT - 1),
                        )
                    pp_sb = smallp.tile([16, NBW], F32R, tag="pp")
                    nc.vector.tensor_copy(pp_sb[:], pool_ps[:].bitcast(F32R))

                    # --- PE-transpose poolPh 128-col slices -> [128,16] ---
                    tr_ps = ps_tr.tile([P, 32], F32, tag="trps")
                    for i in range(2):
                        nc.tensor.transpose(
                            tr_ps[:, 16 * i : 16 * (i + 1)].bitcast(F32R),
                            pp_sb[:, P * i : P * (i + 1)],
                            ident[:],
                        )
                    pht_sb = smallp.tile([P, 32], F32R, tag="pht")
                    nc.vector.tensor_copy(pht_sb[:], tr_ps[:].bitcast(F32R))

                    # --- miniW2: pooled += poolPhT_chunk^T @ W2_chunk ---
                    po_ps = ps_out.tile([16, C], F32, tag="pops")
                    for hh in range(2):
                        for i in range(2):
                            nc.tensor.matmul(
                                po_ps[:, 512 * hh : 512 * (hh + 1)],
                                pht_sb[:, 16 * i : 16 * (i + 1)],
                                w2_sb[:, i, 512 * hh : 512 * (hh + 1)],
                                start=(i == 0),
                                stop=(i == 1),
                            )
                    if nb == 0:
                        nc.vector.tensor_copy(out_sb[:], po_ps[:])
                    else:
                        nc.vector.tensor_add(out_sb[:], out_sb[:], po_ps[:])

                nc.sync.dma_start(out[c], out_sb[:])

    nc.finalize()
    return nc


_CACHED_NC = None


def _get_nc():
    global _CACHED_NC
    if _CACHED_NC is None:
        _CACHED_NC = build_nc()
    return _CACHED_NC


def prepare_inputs(inputs):
    """Host-side sharding: gather selT, build pooling matrices, view weights.

    Returns (in_maps, b2_corrections) where b2_corrections[g] is the host-side
    rank-1 term cnt_t (x) b2_g / TOK to add for nonzero b2 (zero here).
    """
    x = np.ascontiguousarray(np.asarray(inputs["x"], dtype=np.float32))
    nm = np.asarray(inputs["padded_node_mask"])
    em = np.asarray(inputs["padded_edge_mask"])
    ridx = np.asarray(inputs["rand_indices"])

    node_W1 = np.asarray(inputs["node_W1"], dtype=np.float32)
    node_W2 = np.asarray(inputs["node_W2"], dtype=np.float32)
    edge_W1 = np.asarray(inputs["edge_W1"], dtype=np.float32)
    edge_W2 = np.asarray(inputs["edge_W2"], dtype=np.float32)
    for bname in ("node_b1", "node_b2", "edge_b1", "edge_b2"):
        b = np.asarray(inputs[bname])
        if bname.endswith("b1") and np.any(b):
            raise NotImplementedError("nonzero b1 not supported by this kernel")

    xf = x.reshape(T * TOK, D)
    nt, ntok = np.nonzero(nm)
    et, etok = np.nonzero(em)
    assert nt.size == NN and et.size == NN, "unexpected mask population"
    flat_n = nt * TOK + ntok
    flat_e = et * TOK + etok

    in_maps = []
    b2_corr = np.zeros((2 * K, T, C), np.float32)
    any_b2 = np.any(inputs["node_b2"]) or np.any(inputs["edge_b2"])
    for core in range(NCORES):
        sel_list, pt_list = [], []
        for j in range(CPC):
            g = core * CPC + j
            if g < K:
                flat, seg, b2 = flat_n, nt, np.asarray(inputs["node_b2"])[g]
            else:
                flat, seg, b2 = flat_e, et, np.asarray(inputs["edge_b2"])[g - K]
            idx = ridx[g]
            selT_g = np.ascontiguousarray(xf[flat[idx]].T).reshape(KD, P, L)
            sel_list.append(selT_g)
            pt_mat = np.zeros((L, 16), np.float32)
            _, first = np.unique(idx, return_index=True)
            tvals = seg[idx[first]]
            pt_mat[first, tvals] = 1.0 / TOK
            pt_list.append(pt_mat.reshape(MT, MW, 16).transpose(1, 0, 2))
            if any_b2:
                cnt = np.bincount(tvals, minlength=T).astype(np.float32)
                b2_corr[g] = np.outer(cnt / TOK, b2.astype(np.float32))
        if core * CPC < K:
            w1v = node_W1[core * CPC : core * CPC + CPC].reshape(CPC, KD, P, H)
            w2v = node_W2[core * CPC : core * CPC + CPC].reshape(CPC, KH, P, C)
        else:
            o = core * CPC - K
            w1v = edge_W1[o : o + CPC].reshape(CPC, KD, P, H)
            w2v = edge_W2[o : o + CPC].reshape(CPC, KH, P, C)
        in_maps.append(
            {
                "selT": np.ascontiguousarray(np.stack(sel_list)),
                "w1": w1v,
                "w2": w2v,
                "pt": np.ascontiguousarray(np.stack(pt_list)),
            }
        )
    return in_maps, b2_corr


def assemble_output(results, b2_corr):
    comp_all = np.empty((2 * K, T, C), np.float32)
    for core in range(NCORES):
        comp_all[core * CPC : (core + 1) * CPC] = results[core]["out"]
    comp_all += b2_corr
    return np.ascontiguousarray(comp_all.transpose(1, 0, 2).reshape(T, 1, 2 * K * C))


def kernel(**inputs) -> np.ndarray:
    in_maps, b2_corr = prepare_inputs(inputs)
    nc = _get_nc()
    res = run_bass_kernel_spmd(nc, in_maps, list(range(NCORES)))
    return assemble_output(res.results, b2_corr)
